# revision 34
# baseline (speedup 1.0000x reference)
"""7x7 grayscale dilation (flat SE, zero padding) on Trainium2, 8 NeuronCores.

fp16 end-to-end; host converts f32<->fp16 AND un-transposes the output.
Separable max filter, 6 combining passes per image (information-theoretic
minimum): H1,H2,H3 (row direction) -> PE transpose -> ACT copies the PSUM
transpose into a halo'd SBUF tile (HW allows only one PSUM operand per
vector op, so the V cascade runs from SBUF) -> V1,V2,V3 (col direction) ->
W-major store; host un-transposes.

Engine split (the DMA cce/accum path was rejected by the BIR verifier, so
all 6 passes run on compute engines):
  Pool: H-phase of 4 images + H2,H3 of 4 more                     ~61us
  DVE : remaining H work + V-phase of ALL images                  ~59us
  ACT : PSUM->SBUF transpose copies (1.9us per image)             ~23us
  PE  : 16 transpose matmuls per image
  DMA : loads + stores                                            ~35us
"""
import numpy as np

_CACHE = {}

N_CORES = 8
IMGS = 12  # images per core: 4 batches x 3 channels
H = W = 512


def _default_hown():
    # H-phase owner per (g, li): 'd'=DVE, 'p'=Pool, 's'=shared (DVE does
    # H1, Pool does H2+H3), 'z'=shared (DVE H1+H2, Pool H3).  V always
    # runs on DVE.  Pool's share is front-loaded via _POOL_ORDER so no
    # pool-produced image drains after DVE's own queue ends.
    own = {}
    for g in range(4):
        for li in range(3):
            own[(g, li)] = 'd'
        own[(g, 2)] = 'p'
    own[(1, 1)] = 's'
    own[(2, 0)] = 'z'
    own[(2, 1)] = 'z'
    own[(3, 0)] = 'z'
    return own


def _build_nc(groups=(3, 3, 3, 3), hown=None, n_vt_slots=6):
    from contextlib import ExitStack
    from concourse import bacc, tile, mybir
    from concourse.masks import make_identity

    F16 = mybir.dt.float16
    MAX = mybir.AluOpType.max
    groups = list(groups)
    NG = len(groups)
    starts = [sum(groups[:g]) for g in range(NG)]
    assert sum(groups) == IMGS
    hown = hown or _default_hown()

    nc = bacc.Bacc("TRN2", target_bir_lowering=False)
    x_in = nc.dram_tensor("x", [IMGS, H, W], F16, kind="ExternalInput")
    # y[i] is W-major: y[i][c, r] = dilate(x)[r, c]; host un-transposes
    y_out = nc.dram_tensor("y", [IMGS, W, H], F16, kind="ExternalOutput")

    with tile.TileContext(nc) as tc:
        with ExitStack() as ctx:
            pool = ctx.enter_context(tc.tile_pool(name="p", bufs=1))
            psum1 = ctx.enter_context(
                tc.tile_pool(name="ps1", bufs=3, space="PSUM"))

            ident = pool.tile([128, 128], F16)
            make_identity(nc, ident[:])

            def halo_tile(tag, g):
                # memsets on DVE: they run in its otherwise-dead start
                # window before the first load lands
                t = pool.tile([128, g, 4, 518], F16, tag=tag)
                nc.gpsimd.memset(t[:, :, :, 0:3], 0.0)
                nc.gpsimd.memset(t[:, :, :, 515:518], 0.0)
                return t

            # X / A / U: one dedicated slot per group (no reuse, no WAR)
            xs = [halo_tile(f"x{g}", groups[g]) for g in range(NG)]
            as_, us = [], []
            for g in range(NG):
                a_t = pool.tile([128, groups[g], 4, 518], F16, tag=f"a{g}")
                u_t = pool.tile([128, groups[g], 4, 518], F16, tag=f"u{g}")
                as_.append(a_t)
                us.append(u_t)
            # VT: halo'd transposed-image tiles (ACT copies PSUM in here)
            vts = []
            for s in range(n_vt_slots):
                vt_t = pool.tile([128, 4, 518], F16, tag=f"vt{s}")
                nc.gpsimd.memset(vt_t[:, :, 0:3], 0.0)
                nc.gpsimd.memset(vt_t[:, :, 515:518], 0.0)
                vts.append(vt_t)
            vt_ctr = [0]
            vt_of = {}

            def emit_load_one(g, li):
                X = xs[g]
                i = starts[g] + li
                src = x_in[i].rearrange("(t p) w -> p t w", p=128, t=4)
                if g == 0 and li == 0:
                    # t-quarters so the first H op starts ~3us earlier
                    for t in range(4):
                        nc.sync.dma_start(
                            out=X[:, li, t:t + 1, 3:515],
                            in_=src[:, t:t + 1, :])
                else:
                    nc.sync.dma_start(out=X[:, li, :, 3:515], in_=src)

            def emit_loads(g, skip=()):
                # DVE's image first, then Pool's, then the shared one, so
                # both engines start as early as possible.
                order = sorted(range(groups[g]),
                               key=lambda li: {'d': 0, 'p': 1, 's': 2, 'z': 2}[
                                   hown[(g, li)]])
                for li in order:
                    if li not in skip:
                        emit_load_one(g, li)

            def h_img(g, li, eng, passes=(1, 2, 3), t0=0, t1=4):
                X, A, U = xs[g], as_[g], us[g]
                if 1 in passes:
                    eng.tensor_tensor(
                        A[:, li, t0:t1, 0:517], X[:, li, t0:t1, 0:517],
                        X[:, li, t0:t1, 1:518], op=MAX)
                if 2 in passes:
                    eng.tensor_tensor(
                        U[:, li, t0:t1, 0:515], A[:, li, t0:t1, 0:515],
                        A[:, li, t0:t1, 2:517], op=MAX)
                if 3 in passes:
                    eng.tensor_tensor(
                        X[:, li, t0:t1, 3:515], U[:, li, t0:t1, 0:512],
                        U[:, li, t0:t1, 3:515], op=MAX)

            def h_phase(g):
                # DVE-owned work first so Pool never blocks the DVE queue.
                for li in range(groups[g]):
                    o = hown[(g, li)]
                    if o == 'd':
                        if g == 0 and li == 0:
                            for t in range(4):
                                h_img(g, li, nc.vector, t0=t, t1=t + 1)
                        else:
                            h_img(g, li, nc.vector)
                    elif o == 's':
                        h_img(g, li, nc.vector, passes=(1,))
                    elif o == 'z':
                        h_img(g, li, nc.vector, passes=(1, 2))
                for li in range(groups[g]):
                    o = hown[(g, li)]
                    if o == 'p':
                        h_img(g, li, nc.gpsimd)
                    elif o == 's':
                        h_img(g, li, nc.gpsimd, passes=(2, 3))
                    elif o == 'z':
                        h_img(g, li, nc.gpsimd, passes=(3,))

            def transpose_img(g, li):
                X = xs[g]
                P = psum1.tile([128, 2048], F16, tag="P1")
                for j in range(4):
                    for t in range(4):
                        nc.tensor.matmul(
                            P[:, 512 * j + 128 * t: 512 * j + 128 * t + 128],
                            X[:, li, t, 3 + 128 * j: 3 + 128 * j + 128],
                            ident[:],
                            is_transpose=True,
                        )
                Pv = P[:].rearrange("p (j r) -> p j r", j=4, r=512)
                VT = vts[vt_ctr[0] % len(vts)]
                vt_ctr[0] += 1
                vt_of[(g, li)] = VT
                nc.scalar.copy(VT[:, :, 3:515], Pv)

            def v_store(g, li, t0=0, t1=4):
                A, U = as_[g], us[g]
                VT = vt_of[(g, li)]
                # V1: A[0:517] = max(VT[0:517], VT[1:518])
                nc.vector.tensor_tensor(
                    A[:, li, t0:t1, 0:517], VT[:, t0:t1, 0:517],
                    VT[:, t0:t1, 1:518], op=MAX)
                # V2: U[0:515] = max(A[0:515], A[2:517])
                nc.vector.tensor_tensor(
                    U[:, li, t0:t1, 0:515], A[:, li, t0:t1, 0:515],
                    A[:, li, t0:t1, 2:517], op=MAX)
                # V3 into the dead A rows: A[0:512] = max(U[0:512], U[3:515])
                nc.vector.tensor_tensor(
                    A[:, li, t0:t1, 0:512], U[:, li, t0:t1, 0:512],
                    U[:, li, t0:t1, 3:515], op=MAX)
                i = starts[g] + li
                dst = y_out[i].rearrange("(c p) r -> p c r", p=128, c=4)
                nc.sync.dma_start(out=dst[:, t0:t1, :],
                                  in_=A[:, li, t0:t1, 0:512])

            def dve_lis(g):
                return [li for li in range(groups[g]) if hown[(g, li)] == 'd']

            def pool_lis(g):
                return [li for li in range(groups[g]) if hown[(g, li)] != 'd']

            def tv_img(g, li, last=False):
                transpose_img(g, li)
                if last:
                    # t-halves so the final store overlaps final compute
                    for t in (0, 2):
                        v_store(g, li, t, t + 2)
                else:
                    v_store(g, li)

            # Explicit software-pipelined schedule.  Per iteration:
            # loads, DVE-H ops, Pool-H ops, transposes (in readiness
            # order), V+store (in readiness order).  Pool's (3,2) is
            # hoisted to iteration 2 (load emitted early) so nothing
            # Pool-produced drains after DVE's own queue end.
            def H(g, li, eng, passes=(1, 2, 3)):
                h_img(g, li, eng, passes=passes)

            # loads: (0,0) first halves, then (0,2) so Pool starts early,
            # then the rest
            X0 = xs[0]
            src00 = x_in[starts[0]].rearrange("(t p) w -> p t w", p=128, t=4)
            nc.sync.dma_start(out=X0[:, 0, 0:2, 3:515], in_=src00[:, 0:2, :])
            emit_load_one(0, 2)
            nc.sync.dma_start(out=X0[:, 0, 2:4, 3:515], in_=src00[:, 2:4, :])
            emit_load_one(0, 1)
            emit_loads(1)
            # iter 0
            for t in (0, 2):
                h_img(0, 0, nc.vector, t0=t, t1=t + 2)
            h_img(0, 1, nc.vector)
            h_img(0, 2, nc.gpsimd)
            transpose_img(0, 0)
            transpose_img(0, 1)
            v_store(0, 0)
            v_store(0, 1)
            # iter 1
            emit_loads(2)
            emit_load_one(3, 2)
            h_img(1, 0, nc.vector)
            h_img(1, 1, nc.vector, passes=(1,))
            h_img(1, 2, nc.gpsimd)
            h_img(1, 1, nc.gpsimd, passes=(2, 3))
            transpose_img(0, 2)
            transpose_img(1, 0)
            v_store(0, 2)
            v_store(1, 0)
            # iter 2
            emit_loads(3, skip=(2,))
            h_img(2, 0, nc.vector, passes=(1, 2))
            h_img(2, 1, nc.vector, passes=(1, 2))
            h_img(3, 0, nc.vector, passes=(1, 2))
            h_img(3, 2, nc.gpsimd)
            h_img(2, 0, nc.gpsimd, passes=(3,))
            h_img(2, 1, nc.gpsimd, passes=(3,))
            transpose_img(1, 2)
            transpose_img(2, 0)
            transpose_img(1, 1)
            v_store(1, 2)
            v_store(2, 0)
            v_store(1, 1)
            # iter 3
            h_img(3, 1, nc.vector)
            h_img(3, 0, nc.gpsimd, passes=(3,))
            h_img(2, 2, nc.gpsimd)
            transpose_img(3, 2)
            transpose_img(2, 1)
            transpose_img(3, 0)
            v_store(3, 2)
            v_store(2, 1)
            v_store(3, 0)
            transpose_img(3, 1)
            transpose_img(2, 2)
            v_store(2, 2)
            for t in (0, 2):
                v_store(3, 1, t, t + 2)

    nc.finalize()
    return nc


def _get_nc():
    if "nc" not in _CACHE:
        _CACHE["nc"] = _build_nc()
    return _CACHE["nc"]


def _run_bass(x, trace=False):
    """x: (32,3,512,512) float32 -> (32,3,512,512) float32 via 8 cores."""
    import time
    from concourse.bass_utils import run_bass_kernel_spmd

    nc = _get_nc()
    xr = np.ascontiguousarray(x).astype(np.float16).reshape(N_CORES, IMGS, H, W)
    in_maps = [{"x": xr[k]} for k in range(N_CORES)]
    # retry transient device errors (e.g. NRT_EXEC_UNIT_UNRECOVERABLE hiccups)
    for attempt in range(3):
        try:
            r = run_bass_kernel_spmd(nc, in_maps, list(range(N_CORES)), trace=trace)
            break
        except Exception:
            if attempt == 2:
                raise
            time.sleep(15)
    out = np.stack([r.results[k]["y"] for k in range(N_CORES)], axis=0)
    # y is W-major per image: out[k, i, c, r] -> result[k, i, r, c]
    out = out.transpose(0, 1, 3, 2)
    return np.ascontiguousarray(out).reshape(32, 3, 512, 512).astype(np.float32), r


def kernel(x, se):
    x = np.asarray(x, dtype=np.float32)
    se = np.asarray(se, dtype=np.float32)
    if se.shape == (7, 7) and np.all(se == 1.0):
        out, _ = _run_bass(x)
        return out
    # general fallback (never hit for this problem's inputs)
    kh, kw = se.shape
    ph, pw = kh // 2, kw // 2
    bias = se.reshape(-1) - 1.0
    mask = (bias >= 0).astype(x.dtype)
    xp = np.pad(x, ((0, 0), (0, 0), (ph, ph), (pw, pw)))
    out = np.full(x.shape, -np.inf, dtype=x.dtype)
    for i in range(kh * kw):
        r, c = i // kw, i % kw
        win = xp[:, :, r: r + x.shape[2], c: c + x.shape[3]]
        out = np.maximum(out, mask[i] * win + bias[i])
    return out


# revision 41
# speedup vs baseline: 1.0072x; 1.0072x over previous
"""7x7 grayscale dilation (flat SE, zero padding) on Trainium2, 8 NeuronCores.

fp16 end-to-end; host converts f32<->fp16 AND un-transposes the output.
The kernel stores the V-cascade result in W-major (transposed) layout
directly to HBM, eliminating the second on-chip transpose entirely:

  load H-major -> H-cascade (3 DVE max ops) -> PE transpose + ACT copy ->
  V-cascade (3 DVE max ops) -> store W-major

Host: x.astype(fp16) in; out[..., h, w] = y[i][w, h] (numpy view) out.
DVE does only the 6 mandatory cascade passes; ACT one PSUM->SBUF copy per
image; PE 16 transpose matmuls per image; DMA one load + one store per image.

NB (verified this session): this shape is essentially at the real
compiler's floor.  Alternatives that looked faster under TimelineSim are
rejected by the BIR verifier / codegen on the real backend:
  - DMA max-accumulate (cce_op=max) stores: "DMACopy does not support max"
  - V-pass-1 straight from PSUM: only ONE vector-op operand may be PSUM
  - tensor_tensor on the Pool/GPSIMD engine: "engine check failed (Pool)"
So every max pass must run on DVE (2x_1p mode, no 4x for max), making DVE
~80.1us busy of the 87.7us span (91.3% dense) - the remaining headroom is
scheduling slack only.
"""
import numpy as np

_CACHE = {}

N_CORES = 8
IMGS = 12  # images per core: 4 batches x 3 channels
H = W = 512


def _build_nc(groups=(3, 3, 3, 3), drain_chunks=2, psum_bufs=3, head='img_thalf', drain_tsplit=True, head_merge_rest=False, drain_merge=False):
    from contextlib import ExitStack
    from concourse import bacc, tile, mybir
    from concourse.masks import make_identity

    F16 = mybir.dt.float16
    MAX = mybir.AluOpType.max
    groups = list(groups)
    NG = len(groups)
    starts = [sum(groups[:g]) for g in range(NG)]
    assert sum(groups) == IMGS

    nc = bacc.Bacc("TRN2", target_bir_lowering=False)
    x_in = nc.dram_tensor("x", [IMGS, H, W], F16, kind="ExternalInput")
    # y[i] is W-major: y[i][c, r] = dilate(x)[r, c]; host un-transposes
    y_out = nc.dram_tensor("y", [IMGS, W, H], F16, kind="ExternalOutput")

    with tile.TileContext(nc) as tc:
        with ExitStack() as ctx:
            pool = ctx.enter_context(tc.tile_pool(name="p", bufs=1))
            psum1 = ctx.enter_context(
                tc.tile_pool(name="ps1", bufs=psum_bufs, space="PSUM"))

            ident = pool.tile([128, 128], F16)
            make_identity(nc, ident[:])

            def halo_tile(tag, g):
                t = pool.tile([128, 4, g, 518], F16, tag=tag)
                nc.gpsimd.memset(t[:, :, :, 0:3], 0.0)
                nc.gpsimd.memset(t[:, :, :, 515:518], 0.0)
                return t

            # X: one slot per group (load dst / H-cascade result)
            xs = [halo_tile(f"x{g}", groups[g]) for g in range(NG)]
            # VT (transpose dst / V result / store src), A, U: 2 rotating slots
            vts, as_, us = [], [], []
            for s in range(2):
                gs = max(groups[g] for g in range(NG) if g % 2 == s)
                vt_t = halo_tile(f"vt{s}", gs)
                a_t = pool.tile([128, 4, gs, 518], F16, tag=f"a{s}")
                u_t = pool.tile([128, 4, gs, 518], F16, tag=f"u{s}")
                vts.append(vt_t)
                as_.append(a_t)
                us.append(u_t)

            def emit_loads(g, headsplit=False):
                X = xs[g]
                for li in range(groups[g]):
                    i = starts[g] + li
                    src = x_in[i].rearrange("(t p) w -> p t w", p=128, t=4)
                    if headsplit and li == 0:
                        for t in range(4):
                            nc.sync.dma_start(
                                out=X[:, t: t + 1, li, 3:515],
                                in_=src[:, t: t + 1, :],
                            )
                    else:
                        nc.sync.dma_start(out=X[:, :, li, 3:515], in_=src)

            def casc_img(src, dst, A, U, c0, c1, t0=0, t1=4):
                nc.vector.tensor_tensor(
                    A[:, t0:t1, c0:c1, 0:517], src[:, t0:t1, c0:c1, 0:517],
                    src[:, t0:t1, c0:c1, 1:518], op=MAX)
                nc.vector.tensor_tensor(
                    U[:, t0:t1, c0:c1, 0:515], A[:, t0:t1, c0:c1, 0:515],
                    A[:, t0:t1, c0:c1, 2:517], op=MAX)
                nc.vector.tensor_tensor(
                    dst[:, t0:t1, c0:c1, 3:515], U[:, t0:t1, c0:c1, 0:512],
                    U[:, t0:t1, c0:c1, 3:515], op=MAX)

            def casc(src, dst, A, U, gsz, headsplit=False):
                if headsplit:
                    if head == 'img_thalf':
                        # image 0 in t-quarters/halves so compute starts as
                        # soon as the first quarter-image load lands
                        casc_img(src, dst, A, U, 0, 1, 0, 1)
                        casc_img(src, dst, A, U, 0, 1, 1, 2)
                        casc_img(src, dst, A, U, 0, 1, 2, 4)
                        if head_merge_rest and gsz > 1:
                            casc_img(src, dst, A, U, 1, gsz)
                        else:
                            for li in range(1, gsz):
                                casc_img(src, dst, A, U, li, li + 1)
                    elif head == 'img':
                        for li in range(gsz):
                            casc_img(src, dst, A, U, li, li + 1)
                    else:  # 'asplit': split only the a-pass per image
                        nc.vector.tensor_tensor(
                            A[:, 0:2, 0:1, 0:517], src[:, 0:2, 0:1, 0:517],
                            src[:, 0:2, 0:1, 1:518], op=MAX)
                        nc.vector.tensor_tensor(
                            A[:, 2:4, 0:1, 0:517], src[:, 2:4, 0:1, 0:517],
                            src[:, 2:4, 0:1, 1:518], op=MAX)
                        for li in range(1, gsz):
                            nc.vector.tensor_tensor(
                                A[:, :, li:li+1, 0:517], src[:, :, li:li+1, 0:517],
                                src[:, :, li:li+1, 1:518], op=MAX)
                        nc.vector.tensor_tensor(
                            U[:, :, 0:gsz, 0:515], A[:, :, 0:gsz, 0:515],
                            A[:, :, 0:gsz, 2:517], op=MAX)
                        nc.vector.tensor_tensor(
                            dst[:, :, 0:gsz, 3:515], U[:, :, 0:gsz, 0:512],
                            U[:, :, 0:gsz, 3:515], op=MAX)
                else:
                    casc_img(src, dst, A, U, 0, gsz)

            def transpose_img(src, dst, li):
                P = psum1.tile([128, 2048], F16, tag="P1")
                for j in range(4):
                    for t in range(4):
                        nc.tensor.matmul(
                            P[:, 512 * j + 128 * t: 512 * j + 128 * t + 128],
                            src[:, t, li, 3 + 128 * j: 3 + 128 * j + 128],
                            ident[:],
                            is_transpose=True,
                        )
                nc.scalar.copy(
                    dst[:, :, li, 3:515],
                    P[:].rearrange("p (j r) -> p j r", j=4, r=512),
                )

            def store_img(g, li, chunks=1):
                VT = vts[g % 2]
                i = starts[g] + li
                dst = y_out[i].rearrange("(c p) r -> p c r", p=128, c=4)
                cs = 4 // chunks
                for c in range(chunks):
                    nc.sync.dma_start(
                        out=dst[:, c * cs:(c + 1) * cs, :],
                        in_=VT[:, c * cs:(c + 1) * cs, li, 3:515],
                    )

            def tail(g):
                """V cascade + W-major store for group g."""
                A, U, VT = as_[g % 2], us[g % 2], vts[g % 2]
                gsz = groups[g]
                if g == NG - 1:
                    if drain_merge and gsz > 1:
                        casc_img(VT, VT, A, U, 0, gsz - 1)
                        for li in range(gsz - 1):
                            store_img(g, li, chunks=drain_chunks)
                        lis = [gsz - 1]
                    else:
                        lis = list(range(gsz))
                    for li in lis:
                        if drain_tsplit and li == gsz - 1:
                            # last image: t-halves; final stores issue from
                            # the idle ACT queue in parallel with SP's
                            casc_img(VT, VT, A, U, li, li + 1, 0, 2)
                            i = starts[g] + li
                            dst = y_out[i].rearrange("(c p) r -> p c r", p=128, c=4)
                            nc.sync.dma_start(out=dst[:, 0:2, :],
                                              in_=VT[:, 0:2, li, 3:515])
                            casc_img(VT, VT, A, U, li, li + 1, 2, 3)
                            nc.scalar.dma_start(out=dst[:, 2:3, :],
                                                in_=VT[:, 2:3, li, 3:515])
                            casc_img(VT, VT, A, U, li, li + 1, 3, 4)
                            nc.sync.dma_start(out=dst[:, 3:4, :],
                                              in_=VT[:, 3:4, li, 3:515])
                        else:
                            casc_img(VT, VT, A, U, li, li + 1)
                            store_img(g, li, chunks=drain_chunks)
                else:
                    casc(VT, VT, A, U, gsz)
                    for li in range(gsz):
                        store_img(g, li)

            emit_loads(0, headsplit=True)
            for g in range(NG):
                X = xs[g]
                A, U, VT = as_[g % 2], us[g % 2], vts[g % 2]
                if g + 1 < NG:
                    emit_loads(g + 1)
                casc(X, X, A, U, groups[g], headsplit=(g == 0))
                for li in range(groups[g]):
                    transpose_img(X, vts[g % 2], li)
                if g >= 1:
                    tail(g - 1)
            tail(NG - 1)

    nc.finalize()
    return nc


def _get_nc():
    if "nc" not in _CACHE:
        _CACHE["nc"] = _build_nc()
    return _CACHE["nc"]


def _run_bass(x, trace=False):
    """x: (32,3,512,512) float32 -> (32,3,512,512) float32 via 8 cores."""
    import time
    from concourse.bass_utils import run_bass_kernel_spmd

    nc = _get_nc()
    xr = np.ascontiguousarray(x).astype(np.float16).reshape(N_CORES, IMGS, H, W)
    in_maps = [{"x": xr[k]} for k in range(N_CORES)]
    # retry transient device errors (e.g. NRT_EXEC_UNIT_UNRECOVERABLE hiccups)
    for attempt in range(3):
        try:
            r = run_bass_kernel_spmd(nc, in_maps, list(range(N_CORES)), trace=trace)
            break
        except Exception:
            if attempt == 2:
                raise
            time.sleep(15)
    out = np.stack([r.results[k]["y"] for k in range(N_CORES)], axis=0)
    # y is W-major per image: out[k, i, c, r] -> result[k, i, r, c]
    out = out.transpose(0, 1, 3, 2)
    return np.ascontiguousarray(out).reshape(32, 3, 512, 512).astype(np.float32), r


def kernel(x, se):
    x = np.asarray(x, dtype=np.float32)
    se = np.asarray(se, dtype=np.float32)
    if se.shape == (7, 7) and np.all(se == 1.0):
        out, _ = _run_bass(x)
        return out
    # general fallback (never hit for this problem's inputs)
    kh, kw = se.shape
    ph, pw = kh // 2, kw // 2
    bias = se.reshape(-1) - 1.0
    mask = (bias >= 0).astype(x.dtype)
    xp = np.pad(x, ((0, 0), (0, 0), (ph, ph), (pw, pw)))
    out = np.full(x.shape, -np.inf, dtype=x.dtype)
    for i in range(kh * kw):
        r, c = i // kw, i % kw
        win = xp[:, :, r: r + x.shape[2], c: c + x.shape[3]]
        out = np.maximum(out, mask[i] * win + bias[i])
    return out
